# revision 17
# baseline (speedup 1.0000x reference)
"""Dynamic depthwise-3x3 conv (AClayer) on 8 TRN2 NeuronCores.

Structure: out[n,ch,i,j] = sum_p w[n,ch,p] * xpad[n,ch,i+di(p),j+dj(p)]
where w[n,ch,:] = BN(conv1x1(avgpool16x16(x)))[n,:,ch//16,ch%16].

Sharding: core k = (sample k//2, row-half k%2), all 256 channels.
Two NEFF launches:
  Phase A: each core pools its shard via PE matmuls (conv-before-pool swap:
           sigma_raw[o,s] = sum_c w_conv[o,c] * sum_{8x8} x[c,...]) ->
           sigma_loc [9,128] f32 out.
  Host:    relays the 8 tiny sigma blocks to every core (rotated so each
           core's own sample sits first). No math on host.
  Phase B: each core redundantly computes global BN stats (exact sync-BN;
           eps folded for the un-divided pooling sums), builds per-channel
           9-tap weights, and runs the stencil: channels on partitions,
           PE does rows [0,R_PE) via diagonal-matmul accumulation in PSUM
           (ACT evacuates), DVE does rows [R_PE,64) via tensor_scalar(4x) +
           tensor_tensor(2x) chains in bf16.
"""
import numpy as np
import ml_dtypes

import concourse.bass as bass
import concourse.mybir as mybir
from concourse.bass_utils import run_bass_kernel_spmd

bf16 = ml_dtypes.bfloat16
DT = mybir.dt
Alu = mybir.AluOpType
ActF = mybir.ActivationFunctionType

N_CORES = 8
CORE_IDS = list(range(N_CORES))

C, H, W = 256, 128, 128
RS, CS = 66, 130              # shard rows (with halo), padded cols
SH = RS * CS                  # 8580 elems per channel per shard
R_PE = 40                     # PE-region output rows per channel-block
R_DVE = 64 - R_PE
N_CHUNK = R_PE // 4           # PE chunks (4 rows = 512 cols) per block
XS_LEN = (R_DVE + 2) * CS     # shifted-copy span
EPS_EFF = 4096 * 1e-5         # eps for un-divided (x64) pooling sums
# PE chunk -> psum bank; chunk groups per block; cumulative s_act waits
BANK = [0, 1, 2, 3, 4, 5, 6, 7, 0, 1]
GROUPS = [(0, 1, 2, 3), (4, 5, 6, 7), (8, 9)]
ACT_WAIT = [0, 0, 1, 3, 3, 4]   # s_act threshold before group G may run
# tap order for the DVE region: the 6 even-offset taps direct from X,
# the 3 dj=0 taps (odd offset) via the 1-element-shifted copy XS
DVE_TAPS = [(0, 0), (0, 2), (1, 0), (1, 2), (2, 0), (2, 2)]
XS_TAPS = [(0, 1), (1, 1), (2, 1)]


def _make_identity(eng, ap, sq):
    eng.memset(ap, 0.0)
    eng.affine_select(out=ap, in_=ap, compare_op=Alu.not_equal, fill=1.0,
                      base=0, pattern=[[-1, sq]], channel_multiplier=1)


def build_phase_a():
    nc = bass.Bass()
    x = nc.declare_dram_parameter("x", [C, SH], DT.bfloat16, isOutput=False)
    wt = nc.declare_dram_parameter("wt", [128, 18], DT.bfloat16, isOutput=False)
    sig = nc.declare_dram_parameter("sig", [9, 128], DT.float32, isOutput=True)

    with (
        nc.sbuf_tensor("X", [128, 2 * SH], DT.bfloat16) as X,
        nc.sbuf_tensor("WT", [128, 18], DT.bfloat16) as WT,
        nc.sbuf_tensor("SIG", [9, 128], DT.float32) as SIG,
        nc.psum_tensor("PS", [9, 1024], DT.float32) as PS,
        nc.psum_tensor("PSW", [9, 512], DT.float32) as PSW,
        nc.semaphore("s_in") as s_in,
        nc.semaphore("s_mm") as s_mm,
        nc.semaphore("s_red") as s_red,
        nc.semaphore("s_out") as s_out,
        nc.Block() as block,
    ):
        HL = 33 * CS  # rows 0..32 / 33..65

        @block.sync
        def _(sync):
            sync.dma_start(out=WT[:, :], in_=wt[:, :]).then_inc(s_in, 16)
            for cb in (0, 1):  # h0 chunks first
                sync.dma_start(
                    out=X[:, cb * SH: cb * SH + HL],
                    in_=x[cb * 128:(cb + 1) * 128, 0:HL],
                ).then_inc(s_in, 16)
            for cb in (0, 1):
                sync.dma_start(
                    out=X[:, cb * SH + HL: (cb + 1) * SH],
                    in_=x[cb * 128:(cb + 1) * 128, HL:SH],
                ).then_inc(s_in, 16)
            sync.wait_ge(s_red, 2)
            sync.dma_start(out=sig[:, :], in_=SIG[:, :]).then_inc(s_out, 16)
            sync.wait_ge(s_out, 16)

        @block.tensor
        def _(te):
            te.wait_ge(s_in, 16)
            for _ in range(10):  # HAM warm-up on garbage data
                te.matmul(PSW[:, 0:512], lhsT=WT[:, 0:9], rhs=X[:, 0:512],
                          start=True, stop=True)
            for half in (0, 1):
                te.wait_ge(s_in, 48 if half == 0 else 80)
                for cb in (0, 1):
                    lhsT = WT[:, cb * 9: cb * 9 + 9]
                    for pr in range(4 * half, 4 * half + 4):
                        for dr in range(8):
                            r = 8 * pr + dr + 1
                            rhs = X[:, cb * SH + r * CS + 1:
                                    cb * SH + r * CS + 129]
                            # start=True clears the WHOLE psum bank, so only
                            # the first matmul touching each bank may set it
                            mm = te.matmul(
                                PS[:, pr * 128:(pr + 1) * 128], lhsT=lhsT,
                                rhs=rhs,
                                start=(pr % 4 == 0 and cb == 0 and dr == 0),
                                stop=(pr % 4 == 3 and cb == 1 and dr == 7),
                                skip_group_check=True)
            mm.then_inc(s_mm, 1)

        @block.vector
        def _(ve):
            ve.wait_ge(s_mm, 1)
            src = PS[:, :].rearrange("o (q dc) -> o q dc", dc=8)
            # self-sem hop so SIG's write tail is committed before the
            # out-DMA (sem'd via s_red) reads it
            ve.tensor_reduce(SIG[:, :], src, axis=mybir.AxisListType.X,
                             op=Alu.add).then_inc(s_red, 1)
            ve.wait_ge(s_red, 1)
            ve.nop().then_inc(s_red, 1)

    return nc


def build_phase_b():
    nc = bass.Bass()
    x = nc.declare_dram_parameter("x", [C, SH], DT.bfloat16, isOutput=False)
    sgb = nc.declare_dram_parameter("sgb", [9, 1026], DT.float32,
                                    isOutput=False)
    out = nc.declare_dram_parameter("out", [C, 64 * W], DT.bfloat16,
                                    isOutput=True)

    from contextlib import ExitStack
    with ExitStack() as ctx:
        e = ctx.enter_context
        X = e(nc.sbuf_tensor("X", [128, 2 * SH], DT.bfloat16))
        XS = e(nc.sbuf_tensor("XS", [128, 2 * XS_LEN], DT.bfloat16))
        OA = e(nc.sbuf_tensor("OA", [128, 2 * 8192], DT.bfloat16))
        ACC = e(nc.sbuf_tensor("ACC", [128, R_DVE * 128], DT.bfloat16))
        TMP = e(nc.sbuf_tensor("TMP", [128, R_DVE * 128], DT.bfloat16))
        SGB = e(nc.sbuf_tensor("SGB", [9, 1026], DT.float32))
        SQ = e(nc.sbuf_tensor("SQ", [9, 1024], DT.float32))
        ST = e(nc.sbuf_tensor("ST", [9, 12], DT.float32))
        WSM = e(nc.sbuf_tensor("WSM", [9, 256], DT.float32))
        WCH = e(nc.sbuf_tensor("WCH", [128, 18], DT.float32))
        DIAG = e(nc.sbuf_tensor("DIAG", [128, 18 * 128], DT.bfloat16))
        EYE128 = e(nc.sbuf_tensor("EYE128", [128, 128], DT.bfloat16))
        EYE9 = e(nc.sbuf_tensor("EYE9", [9, 9], DT.float32))
        PT = e(nc.psum_tensor("PT", [128, 4096], DT.float32))
        s_in = e(nc.semaphore("s_in"))
        s_xs = e(nc.semaphore("s_xs"))
        s_eye = e(nc.semaphore("s_eye"))
        s_bn1 = e(nc.semaphore("s_bn1"))
        s_act1 = e(nc.semaphore("s_act1"))
        s_bn2 = e(nc.semaphore("s_bn2"))
        s_tp = e(nc.semaphore("s_tp"))
        s_wch = e(nc.semaphore("s_wch"))
        s_pe = e(nc.semaphore("s_pe"))
        s_act = e(nc.semaphore("s_act"))
        s_dve = e(nc.semaphore("s_dve"))
        s_out = e(nc.semaphore("s_out"))
        s_v = e(nc.semaphore("s_v"))
        s_a = e(nc.semaphore("s_a"))
        block = e(nc.Block())
        AL = 42 * CS  # chunk A rows 0..41 (covers PE region + taps)

        @block.sync
        def _(sync):
            sync.dma_start(out=SGB[:, :], in_=sgb[:, :]).then_inc(s_in, 16)
            for cb in (0, 1):
                sync.dma_start(
                    out=X[:, cb * SH: cb * SH + AL],
                    in_=x[cb * 128:(cb + 1) * 128, 0:AL],
                ).then_inc(s_in, 16)
                sync.dma_start(
                    out=X[:, cb * SH + AL:(cb + 1) * SH],
                    in_=x[cb * 128:(cb + 1) * 128, AL:SH],
                ).then_inc(s_in, 16)
            # output DMAs, gated on compute completion
            sync.wait_ge(s_act, 3)
            sync.dma_start(out=out[0:128, 0:R_PE * 128],
                           in_=OA[:, 0:R_PE * 128]).then_inc(s_out, 16)
            sync.wait_ge(s_dve, 1)
            sync.dma_start(out=out[0:128, R_PE * 128:8192],
                           in_=OA[:, R_PE * 128:8192]).then_inc(s_out, 16)
            sync.wait_ge(s_act, 6)
            sync.dma_start(out=out[128:256, 0:R_PE * 128],
                           in_=OA[:, 8192:8192 + R_PE * 128]
                           ).then_inc(s_out, 16)
            sync.wait_ge(s_dve, 2)
            sync.dma_start(out=out[128:256, R_PE * 128:8192],
                           in_=OA[:, 8192 + R_PE * 128:16384]
                           ).then_inc(s_out, 16)
            sync.wait_ge(s_out, 64)

        @block.gpsimd
        def _(gp):
            # memset -> affine_select is a same-engine RAW on tiny ops:
            # self-semaphore the hop (see BN-chain note in the vector block)
            gp.memset(EYE128[:, :], 0.0)
            gp.memset(EYE9[:, :], 0.0).then_inc(s_eye, 1)
            gp.wait_ge(s_eye, 1)
            gp.affine_select(out=EYE128[:, :], in_=EYE128[:, :],
                             compare_op=Alu.not_equal, fill=1.0, base=0,
                             pattern=[[-1, 128]], channel_multiplier=1)
            gp.affine_select(out=EYE9[:, :], in_=EYE9[:, :],
                             compare_op=Alu.not_equal, fill=1.0, base=0,
                             pattern=[[-1, 9]], channel_multiplier=1)
            gp.nop().then_inc(s_eye, 1)
            for cb in (0, 1):
                gp.wait_ge(s_in, 48 + 32 * cb)  # cb fully loaded
                base = cb * SH + R_PE * CS + 1
                gp.dma_start(
                    out=XS[:, cb * XS_LEN: cb * XS_LEN + XS_LEN - 1],
                    in_=X[:, base: base + XS_LEN - 1],
                ).then_inc(s_xs, 16)

        @block.scalar
        def _(sc):
            # dummy sqrt to trigger the ACT table load early
            sc.activation(ST[:, 7:8], ST[:, 6:7], ActF.Sqrt)
            sc.wait_ge(s_bn1, 1)
            # self-sem hop: let the sqrt's write commit before signaling
            sc.activation(ST[:, 7:8], ST[:, 6:7], ActF.Sqrt).then_inc(s_a, 1)
            sc.wait_ge(s_a, 1)
            sc.nop().then_inc(s_act1, 1)
            # evacuate PE psum groups
            na = 1
            for cb in (0, 1):
                for gi, grp in enumerate(GROUPS):
                    G = 3 * cb + gi
                    sc.wait_ge(s_pe, G + 1)
                    for ch in grp:
                        a = sc.activation(
                            OA[:, cb * 8192 + ch * 512: cb * 8192 + ch * 512 + 512],
                            PT[:, BANK[ch] * 512: BANK[ch] * 512 + 512],
                            ActF.Copy)
                    na += 1
                    a.then_inc(s_a, 1)
                    sc.wait_ge(s_a, na)
                    sc.nop().then_inc(s_act, 1)

        @block.vector
        def _(ve):
            # back-to-back dependent DVE ops on tiny operands race (the next
            # op's reads overlap the previous op's in-flight writes), so the
            # whole BN small-op chain is self-semaphored hop by hop.
            vc = [0]

            def step(ins):
                vc[0] += 1
                ins.then_inc(s_v, 1)
                ve.wait_ge(s_v, vc[0])

            ve.wait_ge(s_in, 16)
            # BN stats over all 4*256 pooled positions (raw sums basis)
            step(ve.tensor_reduce(ST[:, 0:1], SGB[:, 0:1024],
                                  axis=mybir.AxisListType.X, op=Alu.add))
            ve.tensor_tensor(SQ[:, :], SGB[:, 0:1024], SGB[:, 0:1024],
                             Alu.mult)
            step(ve.tensor_reduce(ST[:, 1:2], SQ[:, :],
                                  axis=mybir.AxisListType.X, op=Alu.add))
            step(ve.tensor_scalar(ST[:, 2:3], ST[:, 0:1], 1.0 / 1024, None,
                                  Alu.mult))
            step(ve.tensor_scalar(ST[:, 3:4], ST[:, 1:2], 1.0 / 1024, None,
                                  Alu.mult))
            step(ve.tensor_tensor(ST[:, 4:5], ST[:, 2:3], ST[:, 2:3],
                                  Alu.mult))
            step(ve.tensor_tensor(ST[:, 5:6], ST[:, 3:4], ST[:, 4:5],
                                  Alu.subtract))
            step(ve.tensor_scalar(ST[:, 6:7], ST[:, 5:6], EPS_EFF, None,
                                  Alu.add))
            ve.nop().then_inc(s_bn1, 1)
            ve.wait_ge(s_act1, 1)
            step(ve.reciprocal(ST[:, 8:9], ST[:, 7:8]))
            step(ve.tensor_tensor(ST[:, 9:10], ST[:, 8:9], SGB[:, 1024:1025],
                                  Alu.mult))      # s = invstd * gamma
            step(ve.tensor_tensor(ST[:, 10:11], ST[:, 2:3], ST[:, 9:10],
                                  Alu.mult))
            step(ve.tensor_tensor(ST[:, 11:12], SGB[:, 1025:1026],
                                  ST[:, 10:11], Alu.subtract))  # t
            step(ve.tensor_scalar(WSM[:, :], SGB[:, 0:256], ST[:, 9:10],
                                  ST[:, 11:12], Alu.mult, Alu.add))
            ve.nop().then_inc(s_bn2, 1)
            ve.wait_ge(s_tp, 1)
            ve.tensor_copy(WCH[:, 0:9], PT[:, 0:9])
            step(ve.tensor_copy(WCH[:, 9:18], PT[:, 9:18]))
            ve.wait_ge(s_eye, 2)
            for cb in (0, 1):
                for p in range(9):
                    i = cb * 9 + p
                    ve.tensor_scalar(DIAG[:, i * 128:(i + 1) * 128],
                                     EYE128[:, :], WCH[:, i:i + 1], None,
                                     Alu.mult).then_inc(s_wch, 1)
            # DVE stencil region: rows [R_PE, 64) of each channel block
            for cb in (0, 1):
                ve.wait_ge(s_in, 48 + 32 * cb)
                ve.wait_ge(s_xs, 16 + 16 * cb)
                accv = ACC[:, :].rearrange("p (r c) -> p r c", c=128)
                tmpv = TMP[:, :].rearrange("p (r c) -> p r c", c=128)
                outv = OA[:, cb * 8192 + R_PE * 128: cb * 8192 + 8192]
                outv = outv.rearrange("p (r c) -> p r c", c=128)
                xcb = X[:, cb * SH:(cb + 1) * SH].rearrange(
                    "p (r c) -> p r c", c=CS)
                xscb = XS[:, cb * XS_LEN:(cb + 1) * XS_LEN].rearrange(
                    "p (r c) -> p r c", c=CS)
                n_taps = len(DVE_TAPS) + len(XS_TAPS)
                for i in range(n_taps):
                    if i < len(DVE_TAPS):
                        di, dj = DVE_TAPS[i]
                        tap = xcb[:, R_PE + di: R_PE + di + R_DVE,
                                  dj: dj + 128]
                    else:
                        di, dj = XS_TAPS[i - len(DVE_TAPS)]
                        tap = xscb[:, di: di + R_DVE, 0:128]
                    wsc = WCH[:, cb * 9 + 3 * di + dj: cb * 9 + 3 * di + dj + 1]
                    if i == 0:
                        ve.tensor_scalar(accv, tap, wsc, None, Alu.mult)
                    else:
                        ve.tensor_scalar(tmpv, tap, wsc, None, Alu.mult)
                        dst = outv if i == n_taps - 1 else accv
                        last = ve.tensor_tensor(dst, tmpv, accv, Alu.add)
                # self-sem hop so the OutA write tail is committed before
                # the out-DMA (sem'd via s_dve) reads it
                step(last)
                ve.nop().then_inc(s_dve, 1)

        @block.tensor
        def _(te):
            te.wait_ge(s_eye, 2)
            for _ in range(9):  # HAM warm-up
                te.matmul(PT[:, 3584:4096], lhsT=EYE128[:, :],
                          rhs=X[:, 0:512], start=True, stop=True)
            te.wait_ge(s_bn2, 1)
            # both transposes land in psum bank 0: the second must not
            # re-clear the bank (start=True wipes the whole bank)
            te.matmul(PT[:, 0:9], lhsT=WSM[:, 0:128], rhs=EYE9[:, :],
                      is_transpose=True, start=True, stop=False,
                      skip_group_check=True)
            te.matmul(PT[:, 9:18], lhsT=WSM[:, 128:256], rhs=EYE9[:, :],
                      is_transpose=True, start=False, stop=True,
                      skip_group_check=True).then_inc(s_tp, 1)
            for cb in (0, 1):
                te.wait_ge(s_wch, 9 + 9 * cb)
                te.wait_ge(s_in, 32 + 32 * cb)  # chunk A of this cb loaded
                xcb = X[:, cb * SH:(cb + 1) * SH].rearrange(
                    "p (r c) -> p r c", c=CS)
                for gi, grp in enumerate(GROUPS):
                    G = 3 * cb + gi
                    if ACT_WAIT[G]:
                        te.wait_ge(s_act, ACT_WAIT[G])
                    for p in range(9):
                        di, dj = p // 3, p % 3
                        lhsT = DIAG[:, (cb * 9 + p) * 128:
                                    (cb * 9 + p) * 128 + 128]
                        for ch in grp:
                            rhs = xcb[:, 4 * ch + di: 4 * ch + di + 4,
                                      dj: dj + 128]
                            mm = te.matmul(
                                PT[:, BANK[ch] * 512: BANK[ch] * 512 + 512],
                                lhsT=lhsT, rhs=rhs,
                                start=(p == 0), stop=(p == 8))
                    mm.then_inc(s_pe, 1)

    return nc


_CACHE = {}


def kernel(x, w_conv, gamma, beta):
    x = np.asarray(x, dtype=np.float32)
    w_conv = np.asarray(w_conv, dtype=np.float32)
    gamma = np.asarray(gamma, dtype=np.float32)
    beta = np.asarray(beta, dtype=np.float32)
    n = x.shape[0]

    # host-side shard prep (layout only)
    xpad = np.zeros((n, C, H + 2, W + 2), np.float32)
    xpad[:, :, 1:-1, 1:-1] = x
    xb = xpad.astype(bf16)
    shards = [np.ascontiguousarray(
        xb[k // 2, :, 64 * (k % 2):64 * (k % 2) + 66, :]).reshape(C, SH)
        for k in range(N_CORES)]
    wt = np.ascontiguousarray(
        w_conv.reshape(9, 2, 128).transpose(2, 1, 0).reshape(128, 18)
    ).astype(bf16)

    if "A" not in _CACHE:
        _CACHE["A"] = build_phase_a()
        _CACHE["B"] = build_phase_b()

    res_a = run_bass_kernel_spmd(
        _CACHE["A"], [{"x": s, "wt": wt} for s in shards], CORE_IDS)
    sig = np.stack([np.asarray(res_a.results[k]["sig"]) for k in CORE_IDS])
    # sig[k] = raw pooled sigma of (sample k//2, half k%2), [9, 128]
    sig_all = sig.reshape(4, 2, 9, 128).transpose(0, 2, 1, 3).reshape(4, 9, 256)

    in_maps_b = []
    for k in range(N_CORES):
        ni = k // 2
        order = [ni] + [j for j in range(4) if j != ni]
        sgb = np.zeros((9, 1026), np.float32)
        sgb[:, 0:1024] = sig_all[order].transpose(1, 0, 2).reshape(9, 1024)
        sgb[:, 1024] = gamma
        sgb[:, 1025] = beta
        in_maps_b.append({"x": shards[k], "sgb": sgb})

    res_b = run_bass_kernel_spmd(_CACHE["B"], in_maps_b, CORE_IDS)

    outf = np.empty((n, C, H, W), np.float32)
    for k in range(N_CORES):
        o = np.asarray(res_b.results[k]["out"]).reshape(C, 64, W)
        outf[k // 2, :, 64 * (k % 2):64 * (k % 2) + 64, :] = \
            o.astype(np.float32)
    return outf


# revision 22
# speedup vs baseline: 1.0491x; 1.0491x over previous
"""Dynamic depthwise-3x3 conv (AClayer) on 8 TRN2 NeuronCores.

Structure: out[n,ch,i,j] = sum_p w[n,ch,p] * xpad[n,ch,i+di(p),j+dj(p)]
where w[n,ch,:] = BN(conv1x1(avgpool16x16(x)))[n,:,ch//16,ch%16].

Sharding: core k = (sample k//2, row-half k%2), all 256 channels.
Two NEFF launches:
  Phase A: each core pools its shard via PE matmuls (conv-before-pool swap:
           sigma_raw[o,s] = sum_c w_conv[o,c] * sum_{8x8} x[c,...]) ->
           sigma_loc [9,128] f32 out.
  Host:    relays the 8 tiny sigma blocks to every core (rotated so each
           core's own sample sits first). No math on host.
  Phase B: each core redundantly computes global BN stats (exact sync-BN;
           eps folded for the un-divided pooling sums), builds per-channel
           9-tap weights, and runs the stencil: channels on partitions,
           PE does rows [0,R_PE) via diagonal-matmul accumulation in PSUM
           (ACT evacuates), DVE does rows [R_PE,64) via tensor_scalar(4x) +
           tensor_tensor(2x) chains in bf16.
"""
import numpy as np
import ml_dtypes

import concourse.bass as bass
import concourse.mybir as mybir
from concourse.bass_utils import run_bass_kernel_spmd

bf16 = ml_dtypes.bfloat16
DT = mybir.dt
Alu = mybir.AluOpType
ActF = mybir.ActivationFunctionType

N_CORES = 8
CORE_IDS = list(range(N_CORES))

C, H, W = 256, 128, 128
RS, CS = 66, 130              # shard rows (with halo), padded cols
SH = RS * CS                  # 8580 elems per channel per shard
R_PE = 40                     # PE-region output rows per channel-block
R_DVE = 64 - R_PE
N_CHUNK = R_PE // 4           # PE chunks (4 rows = 512 cols) per block
XS_LEN = (R_DVE + 2) * CS     # shifted-copy span
EPS_EFF = 4096 * 1e-5         # eps for un-divided (x64) pooling sums
# PE chunk -> psum bank; chunk groups per block; cumulative s_act waits
BANK = [0, 1, 2, 3, 4, 5, 6, 7, 0, 1]
GROUPS = [(0, 1, 2, 3), (4, 5, 6, 7), (8, 9)]
ACT_WAIT = [0, 0, 1, 3, 3, 4]   # s_act threshold before group G may run
# tap order for the DVE region: the 6 even-offset taps direct from X,
# the 3 dj=0 taps (odd offset) via the 1-element-shifted copy XS
DVE_TAPS = [(0, 0), (0, 2), (1, 0), (1, 2), (2, 0), (2, 2)]
XS_TAPS = [(0, 1), (1, 1), (2, 1)]


def _make_identity(eng, ap, sq):
    eng.memset(ap, 0.0)
    eng.affine_select(out=ap, in_=ap, compare_op=Alu.not_equal, fill=1.0,
                      base=0, pattern=[[-1, sq]], channel_multiplier=1)


def build_phase_a():
    nc = bass.Bass()
    x = nc.declare_dram_parameter("x", [C, SH], DT.bfloat16, isOutput=False)
    wt = nc.declare_dram_parameter("wt", [128, 18], DT.bfloat16, isOutput=False)
    sig = nc.declare_dram_parameter("sig", [9, 128], DT.float32, isOutput=True)

    with (
        nc.sbuf_tensor("X", [128, 2 * SH], DT.bfloat16) as X,
        nc.sbuf_tensor("WT", [128, 18], DT.bfloat16) as WT,
        nc.sbuf_tensor("SIG", [9, 128], DT.float32) as SIG,
        nc.psum_tensor("PS", [9, 1024], DT.float32) as PS,
        nc.psum_tensor("PSW", [9, 512], DT.float32) as PSW,
        nc.semaphore("s_in") as s_in,
        nc.semaphore("s_mm") as s_mm,
        nc.semaphore("s_red") as s_red,
        nc.semaphore("s_out") as s_out,
        nc.Block() as block,
    ):
        HL = 33 * CS  # rows 0..32 / 33..65

        @block.sync
        def _(sync):
            sync.dma_start(out=WT[:, :], in_=wt[:, :]).then_inc(s_in, 16)
            for cb in (0, 1):  # h0 chunks first
                sync.dma_start(
                    out=X[:, cb * SH: cb * SH + HL],
                    in_=x[cb * 128:(cb + 1) * 128, 0:HL],
                ).then_inc(s_in, 16)
            for cb in (0, 1):
                sync.dma_start(
                    out=X[:, cb * SH + HL: (cb + 1) * SH],
                    in_=x[cb * 128:(cb + 1) * 128, HL:SH],
                ).then_inc(s_in, 16)
            sync.wait_ge(s_red, 2)
            sync.dma_start(out=sig[:, :], in_=SIG[:, :]).then_inc(s_out, 16)
            sync.wait_ge(s_out, 16)

        @block.tensor
        def _(te):
            te.wait_ge(s_in, 16)
            for _ in range(10):  # HAM warm-up on garbage data
                te.matmul(PSW[:, 0:512], lhsT=WT[:, 0:9], rhs=X[:, 0:512],
                          start=True, stop=True)
            for half in (0, 1):
                te.wait_ge(s_in, 48 if half == 0 else 80)
                for cb in (0, 1):
                    lhsT = WT[:, cb * 9: cb * 9 + 9]
                    for pr in range(4 * half, 4 * half + 4):
                        for dr in range(8):
                            r = 8 * pr + dr + 1
                            rhs = X[:, cb * SH + r * CS + 1:
                                    cb * SH + r * CS + 129]
                            # start=True clears the WHOLE psum bank, so only
                            # the first matmul touching each bank may set it
                            mm = te.matmul(
                                PS[:, pr * 128:(pr + 1) * 128], lhsT=lhsT,
                                rhs=rhs,
                                start=(pr % 4 == 0 and cb == 0 and dr == 0),
                                stop=(pr % 4 == 3 and cb == 1 and dr == 7),
                                skip_group_check=True)
            mm.then_inc(s_mm, 1)

        @block.vector
        def _(ve):
            ve.wait_ge(s_mm, 1)
            src = PS[:, :].rearrange("o (q dc) -> o q dc", dc=8)
            # self-sem hop so SIG's write tail is committed before the
            # out-DMA (sem'd via s_red) reads it
            ve.tensor_reduce(SIG[:, :], src, axis=mybir.AxisListType.X,
                             op=Alu.add).then_inc(s_red, 1)
            ve.wait_ge(s_red, 1)
            ve.nop().then_inc(s_red, 1)

    return nc


def build_phase_b():
    nc = bass.Bass()
    x = nc.declare_dram_parameter("x", [C, SH], DT.bfloat16, isOutput=False)
    sgb = nc.declare_dram_parameter("sgb", [9, 1026], DT.float32,
                                    isOutput=False)
    out = nc.declare_dram_parameter("out", [C, 64 * W], DT.bfloat16,
                                    isOutput=True)

    from contextlib import ExitStack
    with ExitStack() as ctx:
        e = ctx.enter_context
        X = e(nc.sbuf_tensor("X", [128, 2 * SH], DT.bfloat16))
        XS = e(nc.sbuf_tensor("XS", [128, 2 * XS_LEN], DT.bfloat16))
        OA = e(nc.sbuf_tensor("OA", [128, 2 * 8192], DT.bfloat16))
        ACC = e(nc.sbuf_tensor("ACC", [128, R_DVE * 128], DT.bfloat16))
        TMP = e(nc.sbuf_tensor("TMP", [128, R_DVE * 128], DT.bfloat16))
        SGB = e(nc.sbuf_tensor("SGB", [9, 1026], DT.float32))
        SQ = e(nc.sbuf_tensor("SQ", [9, 1024], DT.float32))
        ST = e(nc.sbuf_tensor("ST", [9, 12], DT.float32))
        WSM = e(nc.sbuf_tensor("WSM", [9, 256], DT.float32))
        WCH = e(nc.sbuf_tensor("WCH", [128, 18], DT.float32))
        DIAG = e(nc.sbuf_tensor("DIAG", [128, 18 * 128], DT.bfloat16))
        EYE128 = e(nc.sbuf_tensor("EYE128", [128, 128], DT.bfloat16))
        EYE9 = e(nc.sbuf_tensor("EYE9", [9, 9], DT.float32))
        PT = e(nc.psum_tensor("PT", [128, 4096], DT.float32))
        s_in = e(nc.semaphore("s_in"))
        s_xs = e(nc.semaphore("s_xs"))
        s_eye = e(nc.semaphore("s_eye"))
        s_bn1 = e(nc.semaphore("s_bn1"))
        s_act1 = e(nc.semaphore("s_act1"))
        s_bn2 = e(nc.semaphore("s_bn2"))
        s_tp = e(nc.semaphore("s_tp"))
        s_wch = e(nc.semaphore("s_wch"))
        s_pe = e(nc.semaphore("s_pe"))
        s_act = e(nc.semaphore("s_act"))
        s_dve = e(nc.semaphore("s_dve"))
        s_out = e(nc.semaphore("s_out"))
        s_v = e(nc.semaphore("s_v"))
        s_a = e(nc.semaphore("s_a"))
        block = e(nc.Block())
        AL = 42 * CS  # chunk A rows 0..41 (covers PE region + taps)

        @block.sync
        def _(sync):
            sync.dma_start(out=SGB[:, :], in_=sgb[:, :]).then_inc(s_in, 16)
            for cb in (0, 1):
                sync.dma_start(
                    out=X[:, cb * SH: cb * SH + AL],
                    in_=x[cb * 128:(cb + 1) * 128, 0:AL],
                ).then_inc(s_in, 16)
                sync.dma_start(
                    out=X[:, cb * SH + AL:(cb + 1) * SH],
                    in_=x[cb * 128:(cb + 1) * 128, AL:SH],
                ).then_inc(s_in, 16)
                # shifted copy for the odd-offset taps, straight from HBM
                # (an SBUF->SBUF copy would crawl behind the HBM loads on
                # the shared SDMA rings)
                sync.dma_start(
                    out=XS[:, cb * XS_LEN: cb * XS_LEN + XS_LEN - 1],
                    in_=x[cb * 128:(cb + 1) * 128,
                          R_PE * CS + 1: R_PE * CS + XS_LEN],
                ).then_inc(s_xs, 16)
            # output DMAs, gated on compute completion
            sync.wait_ge(s_act, 3)
            sync.dma_start(out=out[0:128, 0:R_PE * 128],
                           in_=OA[:, 0:R_PE * 128]).then_inc(s_out, 16)
            sync.wait_ge(s_dve, 1)
            sync.dma_start(out=out[0:128, R_PE * 128:8192],
                           in_=OA[:, R_PE * 128:8192]).then_inc(s_out, 16)
            sync.wait_ge(s_act, 6)
            sync.dma_start(out=out[128:256, 0:R_PE * 128],
                           in_=OA[:, 8192:8192 + R_PE * 128]
                           ).then_inc(s_out, 16)
            sync.wait_ge(s_dve, 2)
            sync.dma_start(out=out[128:256, R_PE * 128:8192],
                           in_=OA[:, 8192 + R_PE * 128:16384]
                           ).then_inc(s_out, 16)
            sync.wait_ge(s_out, 64)

        @block.gpsimd
        def _(gp):
            # memset -> affine_select is a same-engine RAW on tiny ops:
            # self-semaphore the hop (see BN-chain note in the vector block)
            gp.memset(EYE128[:, :], 0.0)
            gp.memset(EYE9[:, :], 0.0).then_inc(s_eye, 1)
            gp.wait_ge(s_eye, 1)
            gp.affine_select(out=EYE128[:, :], in_=EYE128[:, :],
                             compare_op=Alu.not_equal, fill=1.0, base=0,
                             pattern=[[-1, 128]], channel_multiplier=1)
            gp.affine_select(out=EYE9[:, :], in_=EYE9[:, :],
                             compare_op=Alu.not_equal, fill=1.0, base=0,
                             pattern=[[-1, 9]], channel_multiplier=1)
            gp.nop().then_inc(s_eye, 1)

        @block.scalar
        def _(sc):
            # dummy sqrt to trigger the ACT table load early
            sc.activation(ST[:, 7:8], ST[:, 6:7], ActF.Sqrt)
            sc.wait_ge(s_bn1, 1)
            # self-sem hop: let the sqrt's write commit before signaling
            sc.activation(ST[:, 7:8], ST[:, 6:7], ActF.Sqrt).then_inc(s_a, 1)
            sc.wait_ge(s_a, 1)
            sc.nop().then_inc(s_act1, 1)
            # evacuate PE psum groups
            na = 1
            for cb in (0, 1):
                for gi, grp in enumerate(GROUPS):
                    G = 3 * cb + gi
                    sc.wait_ge(s_pe, G + 1)
                    for ch in grp:
                        a = sc.activation(
                            OA[:, cb * 8192 + ch * 512: cb * 8192 + ch * 512 + 512],
                            PT[:, BANK[ch] * 512: BANK[ch] * 512 + 512],
                            ActF.Copy)
                    na += 1
                    a.then_inc(s_a, 1)
                    sc.wait_ge(s_a, na)
                    sc.nop().then_inc(s_act, 1)

        @block.vector
        def _(ve):
            # back-to-back dependent DVE ops on tiny operands race (the next
            # op's reads overlap the previous op's in-flight writes), so the
            # whole BN small-op chain is self-semaphored hop by hop.
            vc = [0]

            def step(ins):
                vc[0] += 1
                ins.then_inc(s_v, 1)
                ve.wait_ge(s_v, vc[0])

            ve.wait_ge(s_in, 16)
            # BN stats over all 4*256 pooled positions (raw sums basis)
            step(ve.tensor_reduce(ST[:, 0:1], SGB[:, 0:1024],
                                  axis=mybir.AxisListType.X, op=Alu.add))
            ve.tensor_tensor(SQ[:, :], SGB[:, 0:1024], SGB[:, 0:1024],
                             Alu.mult)
            step(ve.tensor_reduce(ST[:, 1:2], SQ[:, :],
                                  axis=mybir.AxisListType.X, op=Alu.add))
            step(ve.tensor_scalar(ST[:, 2:3], ST[:, 0:1], 1.0 / 1024, None,
                                  Alu.mult))
            step(ve.tensor_scalar(ST[:, 3:4], ST[:, 1:2], 1.0 / 1024, None,
                                  Alu.mult))
            step(ve.tensor_tensor(ST[:, 4:5], ST[:, 2:3], ST[:, 2:3],
                                  Alu.mult))
            step(ve.tensor_tensor(ST[:, 5:6], ST[:, 3:4], ST[:, 4:5],
                                  Alu.subtract))
            step(ve.tensor_scalar(ST[:, 6:7], ST[:, 5:6], EPS_EFF, None,
                                  Alu.add))
            ve.nop().then_inc(s_bn1, 1)
            ve.wait_ge(s_act1, 1)
            step(ve.reciprocal(ST[:, 8:9], ST[:, 7:8]))
            step(ve.tensor_tensor(ST[:, 9:10], ST[:, 8:9], SGB[:, 1024:1025],
                                  Alu.mult))      # s = invstd * gamma
            step(ve.tensor_tensor(ST[:, 10:11], ST[:, 2:3], ST[:, 9:10],
                                  Alu.mult))
            step(ve.tensor_tensor(ST[:, 11:12], SGB[:, 1025:1026],
                                  ST[:, 10:11], Alu.subtract))  # t
            step(ve.tensor_scalar(WSM[:, :], SGB[:, 0:256], ST[:, 9:10],
                                  ST[:, 11:12], Alu.mult, Alu.add))
            ve.nop().then_inc(s_bn2, 1)
            ve.wait_ge(s_tp, 1)
            ve.tensor_copy(WCH[:, 0:9], PT[:, 0:9])
            step(ve.tensor_copy(WCH[:, 9:18], PT[:, 9:18]))
            ve.wait_ge(s_eye, 2)
            for cb in (0, 1):
                for p in range(9):
                    i = cb * 9 + p
                    ve.tensor_scalar(DIAG[:, i * 128:(i + 1) * 128],
                                     EYE128[:, :], WCH[:, i:i + 1], None,
                                     Alu.mult).then_inc(s_wch, 1)
            # DVE stencil region: rows [R_PE, 64) of each channel block
            for cb in (0, 1):
                ve.wait_ge(s_in, 48 + 32 * cb)
                accv = ACC[:, :].rearrange("p (r c) -> p r c", c=128)
                tmpv = TMP[:, :].rearrange("p (r c) -> p r c", c=128)
                outv = OA[:, cb * 8192 + R_PE * 128: cb * 8192 + 8192]
                outv = outv.rearrange("p (r c) -> p r c", c=128)
                xcb = X[:, cb * SH:(cb + 1) * SH].rearrange(
                    "p (r c) -> p r c", c=CS)
                xscb = XS[:, cb * XS_LEN:(cb + 1) * XS_LEN].rearrange(
                    "p (r c) -> p r c", c=CS)
                n_taps = len(DVE_TAPS) + len(XS_TAPS)
                for i in range(n_taps):
                    if i < len(DVE_TAPS):
                        di, dj = DVE_TAPS[i]
                        tap = xcb[:, R_PE + di: R_PE + di + R_DVE,
                                  dj: dj + 128]
                    else:
                        if i == len(DVE_TAPS):
                            ve.wait_ge(s_xs, 16 + 16 * cb)
                        di, dj = XS_TAPS[i - len(DVE_TAPS)]
                        tap = xscb[:, di: di + R_DVE, 0:128]
                    wsc = WCH[:, cb * 9 + 3 * di + dj: cb * 9 + 3 * di + dj + 1]
                    if i == 0:
                        ve.tensor_scalar(accv, tap, wsc, None, Alu.mult)
                    else:
                        ve.tensor_scalar(tmpv, tap, wsc, None, Alu.mult)
                        dst = outv if i == n_taps - 1 else accv
                        last = ve.tensor_tensor(dst, tmpv, accv, Alu.add)
                # self-sem hop so the OutA write tail is committed before
                # the out-DMA (sem'd via s_dve) reads it
                step(last)
                ve.nop().then_inc(s_dve, 1)

        @block.tensor
        def _(te):
            te.wait_ge(s_eye, 2)
            for _ in range(9):  # HAM warm-up
                te.matmul(PT[:, 3584:4096], lhsT=EYE128[:, :],
                          rhs=X[:, 0:512], start=True, stop=True)
            te.wait_ge(s_bn2, 1)
            # both transposes land in psum bank 0: the second must not
            # re-clear the bank (start=True wipes the whole bank)
            te.matmul(PT[:, 0:9], lhsT=WSM[:, 0:128], rhs=EYE9[:, :],
                      is_transpose=True, start=True, stop=False,
                      skip_group_check=True)
            te.matmul(PT[:, 9:18], lhsT=WSM[:, 128:256], rhs=EYE9[:, :],
                      is_transpose=True, start=False, stop=True,
                      skip_group_check=True).then_inc(s_tp, 1)
            for cb in (0, 1):
                te.wait_ge(s_wch, 9 + 9 * cb)
                te.wait_ge(s_in, 32 + 32 * cb)  # chunk A of this cb loaded
                xcb = X[:, cb * SH:(cb + 1) * SH].rearrange(
                    "p (r c) -> p r c", c=CS)
                for gi, grp in enumerate(GROUPS):
                    G = 3 * cb + gi
                    if ACT_WAIT[G]:
                        te.wait_ge(s_act, ACT_WAIT[G])
                    for p in range(9):
                        di, dj = p // 3, p % 3
                        lhsT = DIAG[:, (cb * 9 + p) * 128:
                                    (cb * 9 + p) * 128 + 128]
                        for ch in grp:
                            rhs = xcb[:, 4 * ch + di: 4 * ch + di + 4,
                                      dj: dj + 128]
                            mm = te.matmul(
                                PT[:, BANK[ch] * 512: BANK[ch] * 512 + 512],
                                lhsT=lhsT, rhs=rhs,
                                start=(p == 0), stop=(p == 8))
                    mm.then_inc(s_pe, 1)

    return nc


_CACHE = {}


def kernel(x, w_conv, gamma, beta):
    x = np.asarray(x, dtype=np.float32)
    w_conv = np.asarray(w_conv, dtype=np.float32)
    gamma = np.asarray(gamma, dtype=np.float32)
    beta = np.asarray(beta, dtype=np.float32)
    n = x.shape[0]

    # host-side shard prep (layout only)
    xpad = np.zeros((n, C, H + 2, W + 2), np.float32)
    xpad[:, :, 1:-1, 1:-1] = x
    xb = xpad.astype(bf16)
    shards = [np.ascontiguousarray(
        xb[k // 2, :, 64 * (k % 2):64 * (k % 2) + 66, :]).reshape(C, SH)
        for k in range(N_CORES)]
    wt = np.ascontiguousarray(
        w_conv.reshape(9, 2, 128).transpose(2, 1, 0).reshape(128, 18)
    ).astype(bf16)

    if "A" not in _CACHE:
        _CACHE["A"] = build_phase_a()
        _CACHE["B"] = build_phase_b()

    res_a = run_bass_kernel_spmd(
        _CACHE["A"], [{"x": s, "wt": wt} for s in shards], CORE_IDS)
    sig = np.stack([np.asarray(res_a.results[k]["sig"]) for k in CORE_IDS])
    # sig[k] = raw pooled sigma of (sample k//2, half k%2), [9, 128]
    sig_all = sig.reshape(4, 2, 9, 128).transpose(0, 2, 1, 3).reshape(4, 9, 256)

    in_maps_b = []
    for k in range(N_CORES):
        ni = k // 2
        order = [ni] + [j for j in range(4) if j != ni]
        sgb = np.zeros((9, 1026), np.float32)
        sgb[:, 0:1024] = sig_all[order].transpose(1, 0, 2).reshape(9, 1024)
        sgb[:, 1024] = gamma
        sgb[:, 1025] = beta
        in_maps_b.append({"x": shards[k], "sgb": sgb})

    res_b = run_bass_kernel_spmd(_CACHE["B"], in_maps_b, CORE_IDS)

    outf = np.empty((n, C, H, W), np.float32)
    for k in range(N_CORES):
        o = np.asarray(res_b.results[k]["out"]).reshape(C, 64, W)
        outf[k // 2, :, 64 * (k % 2):64 * (k % 2) + 64, :] = \
            o.astype(np.float32)
    return outf
